# revision 1
# baseline (speedup 1.0000x reference)
"""CrossInteraction kernel for TRN2, 8-core data parallel.

Math: interaction[b,i,j] = x1[b,i] * x2[b,j]
  mean_dim1[b,i] = x1[b,i] * mean_j(x2[b,j])
  mean_dim2[b,j] = x2[b,j] * mean_i(x1[b,i])
  out = concat([mean_dim1, mean_dim2], axis=1)   # (B, DIM1+DIM2)

The (B, DIM1, DIM2) interaction tensor is never materialized: per batch row
we need one row-mean of x1, one row-mean of x2, and two scaled copies.

Sharding: pure data parallel over batch — 256 rows / 8 cores = 32 rows/core.

Layout: each per-core tensor (32, F) is loaded as a [128, F/4] SBUF tile
with partition = 32*c + b (c = feature-chunk 0..3, b = batch row). This
fills all 128 partitions (full SBUF DMA port bandwidth, 4x DVE lanes).
Row sums are finished with partition-shifted adds that leave the per-row
sum replicated across all 4 chunks' partitions, so the final
tensor_scalar broadcast needs no further shuffling.
"""

import numpy as np

import concourse.bass as bass
import concourse.bacc as bacc
import concourse.tile as tile
from concourse import mybir
from concourse.bass_utils import run_bass_kernel_spmd

BATCH, DIM1, DIM2 = 256, 512, 1024
N_CORES = 8
B_LOC = BATCH // N_CORES  # 32 rows per core
F1 = DIM1 // 4  # 128
F2 = DIM2 // 4  # 256

_FP32 = mybir.dt.float32


def build_nc() -> bass.Bass:
    nc = bacc.Bacc(
        "TRN2", target_bir_lowering=False, debug=False, num_devices=N_CORES
    )
    x1 = nc.dram_tensor("x1", [B_LOC, DIM1], _FP32, kind="ExternalInput").ap()
    x2 = nc.dram_tensor("x2", [B_LOC, DIM2], _FP32, kind="ExternalInput").ap()
    out = nc.dram_tensor("out", [B_LOC, DIM1 + DIM2], _FP32, kind="ExternalOutput").ap()

    # DRAM views matching the [128, F/4] partition=32c+b SBUF layout.
    # DMA pairs src/dst elements in flat enumeration order, so a 3D
    # (c, b, f) DRAM view against a [128, F/4] SBUF tile lands row b's
    # chunk c at partition 32c+b.
    x1_v = x1.rearrange("b (c f) -> c b f", c=4)
    x2_v = x2.rearrange("b (c f) -> c b f", c=4)
    o1_v = out[:, :DIM1].rearrange("b (c f) -> c b f", c=4)
    o2_v = out[:, DIM1:].rearrange("b (c f) -> c b f", c=4)

    with tile.TileContext(nc) as tc:
        with tc.tile_pool(name="p", bufs=1) as pool:
            x1_t = pool.tile([128, F1], _FP32)
            x2_t = pool.tile([128, F2], _FP32)
            nc.sync.dma_start(x1_t[:], x1_v)
            nc.scalar.dma_start(x2_t[:], x2_v)

            # q[:,0] = partial row-sums of x1, q[:,1] = of x2 (per chunk)
            q = pool.tile([128, 2], _FP32)
            nc.vector.reduce_sum(q[:, 0:1], x1_t[:], axis=mybir.AxisListType.X)
            nc.vector.reduce_sum(q[:, 1:2], x2_t[:], axis=mybir.AxisListType.X)

            # Fold the 4 chunk groups (partition p = 32c+b) down to full
            # row sums at partitions 0..31, then replicate back to all 128.
            # Two-SBUF-input ops must share a base partition, so each fold
            # is a partition-shifted copy followed by an aligned add.
            t1 = pool.tile([64, 2], _FP32)
            nc.vector.tensor_copy(t1[:, :], q[64:128, :])
            a = pool.tile([64, 2], _FP32)
            nc.vector.tensor_add(a[:, :], q[0:64, :], t1[:, :])
            t2 = pool.tile([32, 2], _FP32)
            nc.vector.tensor_copy(t2[:, :], a[32:64, :])
            brd = pool.tile([128, 2], _FP32)
            nc.vector.tensor_add(brd[0:32, :], a[0:32, :], t2[:, :])
            nc.vector.tensor_copy(brd[32:64, :], brd[0:32, :])
            nc.vector.tensor_copy(brd[64:128, :], brd[0:64, :])

            # o1 = x1 * mean(x2) ; o2 = x2 * mean(x1)
            o1 = pool.tile([128, F1], _FP32)
            o2 = pool.tile([128, F2], _FP32)
            nc.vector.tensor_scalar(
                o1[:], x1_t[:], brd[:, 1:2], 1.0 / DIM2,
                mybir.AluOpType.mult, mybir.AluOpType.mult,
            )
            nc.vector.tensor_scalar(
                o2[:], x2_t[:], brd[:, 0:1], 1.0 / DIM1,
                mybir.AluOpType.mult, mybir.AluOpType.mult,
            )
            nc.sync.dma_start(o1_v, o1[:])
            nc.scalar.dma_start(o2_v, o2[:])
    nc.compile()
    return nc


def run(x1: np.ndarray, x2: np.ndarray, trace: bool = False):
    """Build + run on 8 cores; returns (full_output, BassKernelResults)."""
    nc = build_nc()
    x1 = np.ascontiguousarray(np.asarray(x1, dtype=np.float32))
    x2 = np.ascontiguousarray(np.asarray(x2, dtype=np.float32))
    in_maps = [
        {
            "x1": x1[i * B_LOC:(i + 1) * B_LOC],
            "x2": x2[i * B_LOC:(i + 1) * B_LOC],
        }
        for i in range(N_CORES)
    ]
    res = run_bass_kernel_spmd(nc, in_maps, list(range(N_CORES)), trace=trace)
    full = np.concatenate([r["out"] for r in res.results], axis=0)
    return full, res


def kernel(x1: np.ndarray, x2: np.ndarray) -> np.ndarray:
    full, _ = run(x1, x2, trace=False)
    return full



# revision 2
# speedup vs baseline: 1.2193x; 1.2193x over previous
"""CrossInteraction kernel for TRN2, 8-core data parallel.

Math: interaction[b,i,j] = x1[b,i] * x2[b,j]
  mean_dim1[b,i] = x1[b,i] * mean_j(x2[b,j])
  mean_dim2[b,j] = x2[b,j] * mean_i(x1[b,i])
  out = concat([mean_dim1, mean_dim2], axis=1)   # (B, DIM1+DIM2)

The (B, DIM1, DIM2) interaction tensor is never materialized: per batch row
we need one row-mean of x1, one row-mean of x2, and two scaled copies.

Sharding: pure data parallel over batch — 256 rows / 8 cores = 32 rows/core.

Layout: each per-core tensor (32, F) is loaded as a [128, F/4] SBUF tile
with partition = 32*c + b (c = feature-chunk 0..3, b = batch row). This
fills all 128 partitions (full DVE lane count), at the cost of needing a
cross-partition fold of the 4 chunk partial sums per row.

Schedule (the whole point of this version):
 - x2 (the bigger, critical input) loads via the SP HWDGE queue; x1 loads
   via the Pool/gpsimd SWDGE queue, so the two descriptor generations run
   in parallel instead of serializing on the single shared HWDGE.
 - The 6-op partition fold chain is replaced by one PE matmul against a
   constant fold-and-broadcast matrix A[p, p'] = (p % 32 == p' % 32),
   built on-chip during the load window (gpsimd iota + 2 DVE ops, all off
   the critical path): brd = A.T @ q replicates each row's 4-chunk sum to
   all 128 partitions in one shot.
 - o1 (small store) is computed first and shipped via SWDGE; o2 rides the
   fast SP HWDGE path, so the two store pipelines overlap.
"""

import numpy as np

import concourse.bass as bass
import concourse.bacc as bacc
import concourse.tile as tile
from concourse import mybir
from concourse.bass_utils import run_bass_kernel_spmd

BATCH, DIM1, DIM2 = 256, 512, 1024
N_CORES = 8
B_LOC = BATCH // N_CORES  # 32 rows per core
F1 = DIM1 // 4  # 128
F2 = DIM2 // 4  # 256

_FP32 = mybir.dt.float32
_I32 = mybir.dt.int32


def build_nc() -> bass.Bass:
    nc = bacc.Bacc(
        "TRN2", target_bir_lowering=False, debug=False, num_devices=N_CORES
    )
    x1 = nc.dram_tensor("x1", [B_LOC, DIM1], _FP32, kind="ExternalInput").ap()
    x2 = nc.dram_tensor("x2", [B_LOC, DIM2], _FP32, kind="ExternalInput").ap()
    out = nc.dram_tensor("out", [B_LOC, DIM1 + DIM2], _FP32, kind="ExternalOutput").ap()

    # DRAM views matching the [128, F/4] partition=32c+b SBUF layout.
    x1_v = x1.rearrange("b (c f) -> c b f", c=4)
    x2_v = x2.rearrange("b (c f) -> c b f", c=4)
    o1_v = out[:, :DIM1].rearrange("b (c f) -> c b f", c=4)
    o2_v = out[:, DIM1:].rearrange("b (c f) -> c b f", c=4)

    with tile.TileContext(nc) as tc:
        with tc.tile_pool(name="p", bufs=1) as pool, \
             tc.tile_pool(name="ps", bufs=1, space="PSUM") as pp:
            x1_t = pool.tile([128, F1], _FP32)
            x2_t = pool.tile([128, F2], _FP32)
            J = pool.tile([128, 128], _I32)
            T = pool.tile([128, 128], _I32)
            A = pool.tile([128, 128], _FP32)
            q1 = pool.tile([128, 1], _FP32)
            q2 = pool.tile([128, 1], _FP32)
            o1 = pool.tile([128, F1], _FP32)
            o2 = pool.tile([128, F2], _FP32)
            s2b = pp.tile([128, 1], _FP32)
            s1b = pp.tile([128, 1], _FP32)

            # Loads on independent DGE paths (HWDGE vs SWDGE).
            nc.sync.dma_start(x2_t[:], x2_v)
            nc.gpsimd.dma_start(x1_t[:], x1_v)

            # Fold matrix A[p, p'] = (p % 32 == p' % 32), built while the
            # loads are in flight. J[p, f] = f - p, then A = ((J & 31) == 0).
            nc.gpsimd.iota(J[:], pattern=[[1, 128]], base=0, channel_multiplier=-1)
            nc.vector.tensor_scalar(T[:], J[:], 31, None, mybir.AluOpType.bitwise_and)
            nc.vector.tensor_scalar(A[:], T[:], 0, None, mybir.AluOpType.is_equal)

            # Per-partition (chunk-partial) row sums.
            nc.vector.reduce_sum(q2[:], x2_t[:], axis=mybir.AxisListType.X)
            nc.vector.reduce_sum(q1[:], x1_t[:], axis=mybir.AxisListType.X)

            # Fold the 4 chunk groups and broadcast to all 128 partitions in
            # a single PE op each: s*b[p'] = sum_p A[p, p'] * q[p].
            nc.tensor.matmul(s2b[:], A[:], q2[:])
            nc.tensor.matmul(s1b[:], A[:], q1[:])

            # o1 = x1 * mean(x2) ; o2 = x2 * mean(x1)  (scale read from PSUM)
            nc.vector.tensor_scalar(
                o1[:], x1_t[:], s2b[:, 0:1], 1.0 / DIM2,
                mybir.AluOpType.mult, mybir.AluOpType.mult,
            )
            nc.vector.tensor_scalar(
                o2[:], x2_t[:], s1b[:, 0:1], 1.0 / DIM1,
                mybir.AluOpType.mult, mybir.AluOpType.mult,
            )

            # Stores on independent DGE paths; o1 is ready first.
            nc.gpsimd.dma_start(o1_v, o1[:])
            nc.sync.dma_start(o2_v, o2[:])
    nc.compile()
    return nc


def run(x1: np.ndarray, x2: np.ndarray, trace: bool = False):
    """Build + run on 8 cores; returns (full_output, BassKernelResults)."""
    nc = build_nc()
    x1 = np.ascontiguousarray(np.asarray(x1, dtype=np.float32))
    x2 = np.ascontiguousarray(np.asarray(x2, dtype=np.float32))
    in_maps = [
        {
            "x1": x1[i * B_LOC:(i + 1) * B_LOC],
            "x2": x2[i * B_LOC:(i + 1) * B_LOC],
        }
        for i in range(N_CORES)
    ]
    res = run_bass_kernel_spmd(nc, in_maps, list(range(N_CORES)), trace=trace)
    full = np.concatenate([r["out"] for r in res.results], axis=0)
    return full, res


def kernel(x1: np.ndarray, x2: np.ndarray) -> np.ndarray:
    full, _ = run(x1, x2, trace=False)
    return full


# revision 3
# speedup vs baseline: 1.2801x; 1.0498x over previous
"""CrossInteraction kernel for TRN2, 8-core data parallel.

Math: interaction[b,i,j] = x1[b,i] * x2[b,j]
  mean_dim1[b,i] = x1[b,i] * mean_j(x2[b,j])
  mean_dim2[b,j] = x2[b,j] * mean_i(x1[b,i])
  out = concat([mean_dim1, mean_dim2], axis=1)   # (B, DIM1+DIM2)

The (B, DIM1, DIM2) interaction tensor is never materialized: per batch row
we need one row-mean of x1, one row-mean of x2, and two scaled copies.

Sharding: pure data parallel over batch — 256 rows / 8 cores = 32 rows/core.

Layout: each per-core tensor (32, F) is loaded as a [128, F/4] SBUF tile
with partition = 32*c + b (c = feature-chunk 0..3, b = batch row). This
fills all 128 partitions (full DVE lane count), at the cost of needing a
cross-partition fold of the 4 chunk partial sums per row.

Schedule (the whole point of this version):
 - x2 (the bigger, critical input) loads via the SP HWDGE queue; x1 loads
   via the Pool/gpsimd SWDGE queue, so the two descriptor generations run
   in parallel instead of serializing on the single shared HWDGE.
 - The 6-op partition fold chain is replaced by one PE matmul against a
   constant fold-and-broadcast matrix A[p, p'] = (p % 32 == p' % 32),
   built on-chip during the load window (gpsimd iota + 2 DVE ops, all off
   the critical path): brd = A.T @ q replicates each row's 4-chunk sum to
   all 128 partitions in one shot.
 - o1 (small store) is computed first and shipped via SWDGE; o2 rides the
   fast SP HWDGE path, so the two store pipelines overlap.
"""

import numpy as np

import concourse.bass as bass
import concourse.bacc as bacc
import concourse.tile as tile
from concourse import mybir
from concourse.bass_utils import run_bass_kernel_spmd

BATCH, DIM1, DIM2 = 256, 512, 1024
N_CORES = 8
B_LOC = BATCH // N_CORES  # 32 rows per core
F1 = DIM1 // 4  # 128
F2 = DIM2 // 4  # 256

_FP32 = mybir.dt.float32
_I32 = mybir.dt.int32


def build_nc() -> bass.Bass:
    nc = bacc.Bacc(
        "TRN2", target_bir_lowering=False, debug=False, num_devices=N_CORES
    )
    # Bass.__init__ unconditionally memsets four const scalar tiles
    # (const-float32-0.0/1.0, const-bfloat16-1.0, const-uint8-127) on the
    # Pool engine before the entry barrier. This kernel never reads them
    # (no activation bias / memset users), but the barrier waits for the
    # memsets, delaying every engine's start by ~370ns. Drop them.
    entry = nc.m.functions[0].blocks[0]
    entry.instructions = [i for i in entry.instructions if i.opcode != "Memset"]
    x1 = nc.dram_tensor("x1", [B_LOC, DIM1], _FP32, kind="ExternalInput").ap()
    x2 = nc.dram_tensor("x2", [B_LOC, DIM2], _FP32, kind="ExternalInput").ap()
    out = nc.dram_tensor("out", [B_LOC, DIM1 + DIM2], _FP32, kind="ExternalOutput").ap()

    # DRAM views matching the [128, F/4] partition=32c+b SBUF layout.
    x1_v = x1.rearrange("b (c f) -> c b f", c=4)
    x2_v = x2.rearrange("b (c f) -> c b f", c=4)
    o1_v = out[:, :DIM1].rearrange("b (c f) -> c b f", c=4)
    o2_v = out[:, DIM1:].rearrange("b (c f) -> c b f", c=4)

    with tile.TileContext(nc) as tc:
        with tc.tile_pool(name="p", bufs=1) as pool, \
             tc.tile_pool(name="ps", bufs=1, space="PSUM") as pp:
            x1_t = pool.tile([128, F1], _FP32)
            x2_t = pool.tile([128, F2], _FP32)
            J = pool.tile([128, 128], _I32)
            T = pool.tile([128, 128], _I32)
            A = pool.tile([128, 128], _FP32)
            q1 = pool.tile([128, 1], _FP32)
            q2 = pool.tile([128, 1], _FP32)
            o1 = pool.tile([128, F1], _FP32)
            o2 = pool.tile([128, F2], _FP32)
            s2b = pp.tile([128, 1], _FP32)
            s1b = pp.tile([128, 1], _FP32)

            # Loads on independent DGE paths (HWDGE vs SWDGE).
            nc.sync.dma_start(x2_t[:], x2_v)
            nc.gpsimd.dma_start(x1_t[:], x1_v)

            # Fold matrix A[p, p'] = (p % 32 == p' % 32), built while the
            # loads are in flight. J[p, f] = f - p, then A = ((J & 31) == 0).
            nc.gpsimd.iota(J[:], pattern=[[1, 128]], base=0, channel_multiplier=-1)
            nc.vector.tensor_scalar(T[:], J[:], 31, None, mybir.AluOpType.bitwise_and)
            nc.vector.tensor_scalar(A[:], T[:], 0, None, mybir.AluOpType.is_equal)

            # Per-partition (chunk-partial) row sums.
            nc.vector.reduce_sum(q2[:], x2_t[:], axis=mybir.AxisListType.X)
            nc.vector.reduce_sum(q1[:], x1_t[:], axis=mybir.AxisListType.X)

            # Fold the 4 chunk groups and broadcast to all 128 partitions in
            # a single PE op each: s*b[p'] = sum_p A[p, p'] * q[p].
            nc.tensor.matmul(s2b[:], A[:], q2[:])
            nc.tensor.matmul(s1b[:], A[:], q1[:])

            # o1 = x1 * mean(x2) ; o2 = x2 * mean(x1)  (scale read from PSUM)
            nc.vector.tensor_scalar(
                o1[:], x1_t[:], s2b[:, 0:1], 1.0 / DIM2,
                mybir.AluOpType.mult, mybir.AluOpType.mult,
            )
            nc.vector.tensor_scalar(
                o2[:], x2_t[:], s1b[:, 0:1], 1.0 / DIM1,
                mybir.AluOpType.mult, mybir.AluOpType.mult,
            )

            # Stores on independent DGE paths; o1 is ready first.
            nc.gpsimd.dma_start(o1_v, o1[:])
            nc.sync.dma_start(o2_v, o2[:])
    nc.compile()
    return nc


def run(x1: np.ndarray, x2: np.ndarray, trace: bool = False):
    """Build + run on 8 cores; returns (full_output, BassKernelResults)."""
    nc = build_nc()
    x1 = np.ascontiguousarray(np.asarray(x1, dtype=np.float32))
    x2 = np.ascontiguousarray(np.asarray(x2, dtype=np.float32))
    in_maps = [
        {
            "x1": x1[i * B_LOC:(i + 1) * B_LOC],
            "x2": x2[i * B_LOC:(i + 1) * B_LOC],
        }
        for i in range(N_CORES)
    ]
    res = run_bass_kernel_spmd(nc, in_maps, list(range(N_CORES)), trace=trace)
    full = np.concatenate([r["out"] for r in res.results], axis=0)
    return full, res


def kernel(x1: np.ndarray, x2: np.ndarray) -> np.ndarray:
    full, _ = run(x1, x2, trace=False)
    return full
